# revision 26
# baseline (speedup 1.0000x reference)
"""PlainGCN message passing on 8 TRN2 NeuronCores.

Computation (reference):
    deg = bincount(h); dis = deg**-0.5; norm = dis[t]*dis[h]
    out = relu(segment_sum(norm[:,None] * x[h], t, N))

Strategy (v4):
  - norm factorizes: out[t] = relu(dis[t] * segsum(dis[h]*x[h])). Pre-scale
    xs = dis[:,None]*x on the host and cast to bf16; per-edge scaling
    disappears and the segment-sum weights are exactly 1.0/0.0.
  - Shard edges by destination node: core c owns dest nodes
    [c*N/8, (c+1)*N/8) and all edges targeting them. xs replicated.
  - dma_gather xs rows by h (int16 indices => 4 equal source buckets of
    25000 rows). Gather descriptor generation runs on GPSIMD Q7 core
    pair `queue_num`; the 4 bucket-gathers of each group use the 4 SWDGE
    queues so generation runs 4-way parallel (~8.5 ns/desc/queue is the
    hard per-pair rate). Tiny warmup gathers pre-load the Q7 IRAM.
  - Edge runs are packed TIGHT (no 64-alignment): descriptor count is
    the wall, so padding is minimized. The matmul "one-hot" blocks are
    built on the HOST per segment (a segment = one full 128-slot column
    of one dest tile's run) and streamed in as a bf16 input over the
    hardware DMA path, which has slack. Every matmul is then a full
    k=128 base-0 column and each dest tile accumulates in ONE PSUM
    chain.
  - Output: relu(dis[t] * psum) fused in one ScalarE activation with a
    per-partition scale; DMA out.
  - SPMD: all 8 cores share one program. Per-(tile,bucket) run lengths
    are padded to the max across cores; pad slots gather row 0 and have
    all-zero one-hot columns, contributing nothing.
"""

import numpy as np

import concourse.bacc as bacc
import concourse.mybir as mybir
import concourse.tile as tile
from concourse.bass_utils import run_bass_kernel_spmd
from concourse.library_config import mlp as mlp_lib

P = 128
BF16 = mybir.dt.bfloat16
FP8 = mybir.dt.float8e4


def _np_bf16():
    return np.dtype(mybir.dt.np(BF16))


def _preprocess(x, h, t, n_cores, n_buckets, tiles_per_group):
    """Host-side edge sharding + schedule + one-hot block construction."""
    n, d = x.shape
    assert n % n_cores == 0
    npc = n // n_cores  # nodes per core
    n_tiles = -(-npc // P)  # dest tiles per core
    assert n % n_buckets == 0
    bucket = n // n_buckets
    assert bucket <= 32767  # int16 gather indices

    h = h.astype(np.int64)
    t = t.astype(np.int64)

    deg = np.bincount(h, minlength=n).astype(np.float32)
    dis = np.where(deg > 0, deg, 1).astype(np.float32) ** np.float32(-0.5)
    xs = (x.astype(np.float32) * dis[:, None]).astype(_np_bf16())

    b = h // bucket
    gidx_all = (h - b * bucket).astype(np.int16)

    # Assign dest nodes to (core, tile-slot, row) bins with a greedy
    # packer so the per-(slot,bucket) max-over-cores edge count — which
    # sets the shared-schedule descriptor total, the hard Q7 floor —
    # approaches the mean. The output y is unpermuted on the host.
    Cm = np.zeros((n, n_buckets), dtype=np.int64)
    np.add.at(Cm, (t, b), 1)
    ndeg = Cm.sum(1)
    cap_slot = np.minimum(P, npc - np.arange(n_tiles) * P)  # rows written
    cap = np.tile(cap_slot, (n_cores, 1))  # [cores, slots]
    bins = np.zeros((n_cores, n_tiles, n_buckets), dtype=np.int64)
    fill = np.zeros((n_cores, n_tiles), dtype=np.int64)
    core_of = np.zeros(n, dtype=np.int64)
    slot_of = np.zeros(n, dtype=np.int64)
    row_of = np.zeros(n, dtype=np.int64)
    for nd in np.argsort(-ndeg, kind="stable"):
        v = Cm[nd]
        rem = cap - fill
        mx = bins.max(0)  # [slots, buckets]
        inc = np.maximum(0, bins + v[None, None, :] - mx[None, :, :]).sum(-1)
        inc = np.where(rem > 0, inc, 1 << 40).astype(np.float64)
        ci, si = np.unravel_index(np.argmin(inc - 0.001 * rem), inc.shape)
        core_of[nd] = ci
        slot_of[nd] = si
        row_of[nd] = fill[ci, si]
        bins[ci, si] += v
        fill[ci, si] += 1

    core = core_of[t]
    j = slot_of[t]  # dest tile slot
    tin = row_of[t]
    run_len = bins.max(axis=0)  # [n_tiles, n_buckets]

    # small leading groups prime the gather/compute pipeline quickly and
    # a small trailing group shortens the tail; large steady-state groups
    # amortize the ~2.3us fixed cost per gather instruction
    sizes = [2, 2, 4]
    while sum(sizes) < n_tiles - 2:
        sizes.append(min(tiles_per_group, n_tiles - 2 - sum(sizes)))
    sizes.append(n_tiles - sum(sizes))
    groups = []
    at = 0
    for s in sizes:
        groups.append(list(range(at, at + s)))
        at += s
    n_groups = len(groups)
    grp_of_tile = np.zeros(n_tiles, dtype=np.int64)
    for g, tiles_g in enumerate(groups):
        for jj in tiles_g:
            grp_of_tile[jj] = g

    # Within a span the packing order of the group's runs is free, and
    # the number of PE matmul segments (columns touched) depends on the
    # cumulative offsets mod 128. The position after packing a subset is
    # determined by its length sum, so an exact DP over bitmasks finds
    # the order minimizing segments — at zero gather cost.
    def _best_order(lens):
        m = len(lens)
        full = 1 << m
        INF = 1 << 30
        cost = [INF] * full
        parent = [-1] * full
        cost[0] = 0
        psum = [0] * full
        for mask in range(1, full):
            lo = mask & -mask
            psum[mask] = psum[mask ^ lo] + lens[lo.bit_length() - 1]
        for mask in range(full):
            c = cost[mask]
            if c == INF:
                continue
            pos = psum[mask] % P
            for i in range(m):
                bit = 1 << i
                if mask & bit:
                    continue
                L = lens[i]
                # columns touched = floor((pos+L-1)/P) - floor(pos/P) + 1
                step = (pos + L - 1) // P - pos // P + 1 if L else 0
                nm = mask | bit
                if c + step < cost[nm]:
                    cost[nm] = c + step
                    parent[nm] = i
        order = []
        mask = full - 1
        while mask:
            i = parent[mask]
            order.append(i)
            mask ^= 1 << i
        order.reverse()
        return order

    # Stream layout: spans (g, b) hold the packed runs of group g's tiles
    # in bucket b, padded to a multiple of P at span end (pad slots
    # gather row 0; their one-hot columns are all-zero).
    spans = []  # (g, b, start, length)
    run_start = np.zeros((n_tiles, n_buckets), dtype=np.int64)
    # seg_lists[jj]: ordered (seg_id, col, b); segments are full columns
    seg_lists = [[] for _ in range(n_tiles)]
    seg_base = np.full((n_tiles, n_buckets), -1, dtype=np.int64)
    pos = 0
    for g, tiles_g in enumerate(groups):
        for bb in range(n_buckets):
            s0 = pos
            lens = [int(run_len[jj, bb]) for jj in tiles_g]
            order = _best_order(lens)
            for ti in order:
                jj = tiles_g[ti]
                run_start[jj, bb] = pos
                pos += lens[ti]
            pos = -(-pos // P) * P
            spans.append((g, bb, s0, pos - s0))
    e_pad = pos
    n_cols = e_pad // P
    n_seg_est = sum(
        (int(run_start[jj, bb]) + int(run_len[jj, bb]) - 1) // P
        - int(run_start[jj, bb]) // P + 1
        for jj in range(n_tiles) for bb in range(n_buckets)
        if run_len[jj, bb]
    )
    print(f"[kernel] e_pad={e_pad} est_segs={n_seg_est}", flush=True)

    # Assign segment ids in (group -> tile -> bucket -> col) order so each
    # group's one-hot blocks are contiguous and PSUM chains are per-tile.
    n_segs = 0
    group_seg = []  # (seg_start, seg_end) per group
    for g, tiles_g in enumerate(groups):
        g0 = n_segs
        for jj in tiles_g:
            for bb in range(n_buckets):
                rl = int(run_len[jj, bb])
                if rl == 0:
                    continue
                rs = int(run_start[jj, bb])
                c0 = rs // P
                c1 = -(-(rs + rl) // P)
                seg_base[jj, bb] = n_segs
                for c in range(c0, c1):
                    seg_lists[jj].append((n_segs, c, bb))
                    n_segs += 1
        group_seg.append((g0, n_segs))

    # Per-core data arrays
    per_core = []
    order_key = (((core * n_groups * n_buckets) + grp_of_tile[j] * n_buckets + b)
                 * n_tiles + j)
    sort_idx = np.argsort(order_key, kind="stable")
    for c in range(n_cores):
        sel = sort_idx[core[sort_idx] == c]
        jj = j[sel]
        bb2 = b[sel]
        # rank within (tile,bucket) run, order of appearance
        key = jj * n_buckets + bb2
        change = np.r_[True, key[1:] != key[:-1]]
        grp_id = np.cumsum(change) - 1
        first_pos = np.nonzero(change)[0]
        within = np.arange(len(sel)) - first_pos[grp_id]
        posn = run_start[jj, bb2] + within

        gi = np.zeros(e_pad, dtype=np.int16)
        gi[posn] = gidx_all[sel]

        # wrap gather indices: per span, index l -> [l%16, l//16], tiled x8
        # (replicated so any Q7 core pair / queue can read its copy)
        wrap = np.zeros((P, e_pad // 16), dtype=np.int16)
        for (_g, _b, s0, ln) in spans:
            w0 = s0 // 16
            seg = gi[s0:s0 + ln].reshape(ln // 16, 16).T  # [16, ln/16]
            wrap[:, w0:w0 + ln // 16] = np.tile(seg, (8, 1))

        # host-built one-hot blocks: oh[p, s*128 + f] = 1 iff this core
        # has an edge at slot (col_of_seg s, partition p) with dest row f
        s_e = seg_base[jj, bb2] + (posn // P - run_start[jj, bb2] // P)
        oh = np.zeros((P, n_segs * P), dtype=np.dtype(mybir.dt.np(FP8)))
        oh[posn % P, s_e * P + tin[sel]] = np.float32(1.0)

        # dis value per dest row (for the output scale stage)
        disF = np.ones((P, n_tiles), dtype=np.float32)
        mine = np.nonzero(core_of == c)[0]
        disF[row_of[mine], slot_of[mine]] = dis[mine]

        per_core.append({"gidx": wrap, "oh": oh, "disv": disF})

    # node -> row of the concatenated per-core y output (for unpermute)
    y_pos = core_of * npc + slot_of * P + row_of

    schedule = {
        "n": n, "d": d, "npc": npc, "n_tiles": n_tiles, "n_cols": n_cols,
        "e_pad": e_pad, "bucket": bucket, "n_buckets": n_buckets,
        "groups": groups, "spans": spans, "seg_lists": seg_lists,
        "n_segs": n_segs, "group_seg": group_seg, "y_pos": y_pos,
    }
    return schedule, per_core, {"xs": xs}


def _build_program(sched, n_cores):
    n, d, npc = sched["n"], sched["d"], sched["npc"]
    n_tiles, e_pad = sched["n_tiles"], sched["e_pad"]
    bucket, n_buckets = sched["bucket"], sched["n_buckets"]
    groups, spans, seg_lists = sched["groups"], sched["spans"], sched["seg_lists"]
    n_segs, group_seg = sched["n_segs"], sched["group_seg"]

    nc = bacc.Bacc("TRN2", target_bir_lowering=False, debug=False,
                   num_devices=n_cores, num_swdge_queues=4,
                   dynamic_dma_scratch_size=49152)
    f32 = mybir.dt.float32
    xs_d = nc.dram_tensor("xs", [n, d], BF16, kind="ExternalInput")
    gidx_d = nc.dram_tensor("gidx", [P, e_pad // 16], mybir.dt.int16,
                            kind="ExternalInput")
    oh_d = nc.dram_tensor("oh", [P, n_segs * P], FP8, kind="ExternalInput")
    disv_d = nc.dram_tensor("disv", [P, n_tiles], f32, kind="ExternalInput")
    y_d = nc.dram_tensor("y", [npc, d], f32, kind="ExternalOutput")

    nc.gpsimd.load_library(mlp_lib)

    max_span = max(ln for (_g, _b, _s, ln) in spans)
    span_by_gb = {(g, b): (s0, ln) for (g, b, s0, ln) in spans}
    group_range = []  # stream range of each group
    for g in range(len(groups)):
        gs0 = span_by_gb[(g, 0)][0]
        last_s0, last_ln = span_by_gb[(g, n_buckets - 1)]
        group_range.append((gs0, last_s0 + last_ln))
    max_gsegs = max(s1 - s0 for (s0, s1) in group_seg)

    relu = mybir.ActivationFunctionType.Relu

    with tile.TileContext(nc) as tc:
        with (
            tc.tile_pool(name="const", bufs=1) as cpool,
            tc.tile_pool(name="gidx", bufs=len(groups) + 1) as ipool,
            tc.tile_pool(name="gather", bufs=20) as gpool,
            tc.tile_pool(name="onehot", bufs=4) as opool,
            tc.tile_pool(name="psum", bufs=8, space="PSUM") as ppool,
            tc.tile_pool(name="outs", bufs=6) as ypool,
        ):
            # Q7 IRAM warmup: one tiny gather per SWDGE queue so the
            # dma_gather ucode is resident on all 4 core pairs before the
            # real gathers arrive.
            widx = cpool.tile([P, 1], mybir.dt.int16, tag="widx")
            nc.vector.memset(widx[:], 0)
            for q in range(4):
                wt = cpool.tile([P, d], BF16, tag=f"warm{q}")
                wt_3d = wt[:, :].rearrange("p (c d) -> p c d", d=d)
                nc.gpsimd.dma_gather(wt_3d, xs_d[0:bucket, :], widx[:, :],
                                     16, 16, d, single_packet=True,
                                     queue_num=q)

            disv_t = cpool.tile([P, n_tiles], f32, tag="disv")
            nc.sync.dma_start(disv_t[:], disv_d[:, :])

            # per-group gather index tiles (first group's load lands fast)
            gidx_tiles = []
            for g, (gs0, gs1) in enumerate(group_range):
                it = ipool.tile([P, (gs1 - gs0) // 16], mybir.dt.int16,
                                tag="gidx", name=f"gidx{g}")
                nc.sync.dma_start(it[:], gidx_d[:, gs0 // 16:gs1 // 16])
                gidx_tiles.append(it)

            for g, tiles_g in enumerate(groups):
                gs0, _gs1 = group_range[g]
                seg0, seg1 = group_seg[g]

                gtiles = {}
                for b in range(n_buckets):
                    s0, ln = span_by_gb[(g, b)]
                    if ln == 0:
                        continue
                    base = b * bucket
                    gt = gpool.tile([P, (max_span // P) * d], BF16, tag="gt",
                                    name=f"gt{g}_{b}")
                    gt_3d = gt[:, :(ln // P) * d].rearrange(
                        "p (c d) -> p c d", d=d
                    )
                    nc.gpsimd.dma_gather(
                        gt_3d,
                        xs_d[base:base + bucket, :],
                        gidx_tiles[g][:, (s0 - gs0) // 16:(s0 - gs0 + ln) // 16],
                        ln, ln, d,
                        single_packet=(ln <= 1024),
                        queue_num=b % 4,
                    )
                    gtiles[b] = (gt, s0)

                # host-built one-hot blocks for this group's segments
                oh = opool.tile([P, max_gsegs * P], FP8, tag="oh",
                                name=f"oh{g}")
                nc.sync.dma_start(oh[:, :(seg1 - seg0) * P],
                                  oh_d[:, seg0 * P:seg1 * P])

                for jj in tiles_g:
                    segs = seg_lists[jj]
                    rows = min(P, npc - jj * P)
                    yt = ypool.tile([P, d], f32, tag="yt", name=f"yt{jj}")
                    if segs:
                        pt = ppool.tile([P, d], f32, tag="ps",
                                        name=f"ps{jj}")
                        for si, (sid, col, b) in enumerate(segs):
                            gt, s0 = gtiles[b]
                            col_l = col - s0 // P
                            nc.tensor.matmul(
                                pt[:],
                                lhsT=oh[:, (sid - seg0) * P:(sid - seg0 + 1) * P],
                                rhs=gt[:, col_l * d:(col_l + 1) * d],
                                start=(si == 0),
                                stop=(si == len(segs) - 1),
                            )
                        nc.scalar.activation(yt[:], pt[:], relu,
                                             scale=disv_t[:, jj:jj + 1])
                    else:
                        nc.vector.memset(yt[:], 0.0)
                    nc.sync.dma_start(y_d[jj * P:jj * P + rows, :],
                                      yt[:rows, :])

    nc.compile()
    return nc


def _run(x, h, t, n_cores=8, n_buckets=4, tiles_per_group=8, trace=False):
    import time
    t0 = time.monotonic()
    sched, per_core, shared = _preprocess(x, h, t, n_cores, n_buckets,
                                          tiles_per_group)
    t1 = time.monotonic()
    print(f"[kernel] preprocess {t1 - t0:.1f}s  e_pad={sched['e_pad']} "
          f"segs={sched['n_segs']}", flush=True)
    nc = _build_program(sched, n_cores)
    t2 = time.monotonic()
    print(f"[kernel] build+tile-schedule {t2 - t1:.1f}s", flush=True)
    in_maps = [
        {"xs": shared["xs"], "gidx": pc["gidx"], "oh": pc["oh"],
         "disv": pc["disv"]}
        for pc in per_core
    ]
    res = run_bass_kernel_spmd(nc, in_maps, core_ids=list(range(n_cores)),
                               trace=trace)
    t3 = time.monotonic()
    print(f"[kernel] compile+run {t3 - t2:.1f}s", flush=True)
    y_cat = np.concatenate([res.results[c]["y"] for c in range(n_cores)],
                           axis=0)
    y = y_cat[sched["y_pos"]]
    return y, res


def kernel(x, h, t):
    y, _ = _run(np.asarray(x), np.asarray(h), np.asarray(t))
    return y


# revision 27
# speedup vs baseline: 1.0662x; 1.0662x over previous
"""PlainGCN message passing on 8 TRN2 NeuronCores.

Computation (reference):
    deg = bincount(h); dis = deg**-0.5; norm = dis[t]*dis[h]
    out = relu(segment_sum(norm[:,None] * x[h], t, N))

Strategy (v4):
  - norm factorizes: out[t] = relu(dis[t] * segsum(dis[h]*x[h])). Pre-scale
    xs = dis[:,None]*x on the host and cast to bf16; per-edge scaling
    disappears and the segment-sum weights are exactly 1.0/0.0.
  - Shard edges by destination node: core c owns dest nodes
    [c*N/8, (c+1)*N/8) and all edges targeting them. xs replicated.
  - dma_gather xs rows by h (int16 indices => 4 equal source buckets of
    25000 rows). Gather descriptor generation runs on GPSIMD Q7 core
    pair `queue_num`; the 4 bucket-gathers of each group use the 4 SWDGE
    queues so generation runs 4-way parallel (~8.5 ns/desc/queue is the
    hard per-pair rate). Tiny warmup gathers pre-load the Q7 IRAM.
  - Edge runs are packed TIGHT (no 64-alignment): descriptor count is
    the wall, so padding is minimized. The matmul "one-hot" blocks are
    built on the HOST per segment (a segment = one full 128-slot column
    of one dest tile's run) and streamed in as a bf16 input over the
    hardware DMA path, which has slack. Every matmul is then a full
    k=128 base-0 column and each dest tile accumulates in ONE PSUM
    chain.
  - Output: relu(dis[t] * psum) fused in one ScalarE activation with a
    per-partition scale; DMA out.
  - SPMD: all 8 cores share one program. Per-(tile,bucket) run lengths
    are padded to the max across cores; pad slots gather row 0 and have
    all-zero one-hot columns, contributing nothing.
"""

import numpy as np

import concourse.bacc as bacc
import concourse.mybir as mybir
import concourse.tile as tile
from concourse.bass_utils import run_bass_kernel_spmd
from concourse.library_config import mlp as mlp_lib

P = 128
BF16 = mybir.dt.bfloat16
FP8 = mybir.dt.float8e4


def _np_bf16():
    return np.dtype(mybir.dt.np(BF16))


def _preprocess(x, h, t, n_cores, n_buckets, tiles_per_group):
    """Host-side edge sharding + schedule + one-hot block construction."""
    n, d = x.shape
    assert n % n_cores == 0
    npc = n // n_cores  # nodes per core
    n_tiles = -(-npc // P)  # dest tiles per core
    assert n % n_buckets == 0
    bucket = n // n_buckets
    assert bucket <= 32767  # int16 gather indices

    h = h.astype(np.int64)
    t = t.astype(np.int64)

    deg = np.bincount(h, minlength=n).astype(np.float32)
    dis = np.where(deg > 0, deg, 1).astype(np.float32) ** np.float32(-0.5)
    xs = (x.astype(np.float32) * dis[:, None]).astype(_np_bf16())

    b = h // bucket
    gidx_all = (h - b * bucket).astype(np.int16)

    # Assign dest nodes to (core, tile-slot, row) bins with a greedy
    # packer so the per-(slot,bucket) max-over-cores edge count — which
    # sets the shared-schedule descriptor total, the hard Q7 floor —
    # approaches the mean. The output y is unpermuted on the host.
    Cm = np.zeros((n, n_buckets), dtype=np.int64)
    np.add.at(Cm, (t, b), 1)
    ndeg = Cm.sum(1)
    cap_slot = np.minimum(P, npc - np.arange(n_tiles) * P)  # rows written
    cap = np.tile(cap_slot, (n_cores, 1))  # [cores, slots]
    bins = np.zeros((n_cores, n_tiles, n_buckets), dtype=np.int64)
    fill = np.zeros((n_cores, n_tiles), dtype=np.int64)
    core_of = np.zeros(n, dtype=np.int64)
    slot_of = np.zeros(n, dtype=np.int64)
    row_of = np.zeros(n, dtype=np.int64)
    for nd in np.argsort(-ndeg, kind="stable"):
        v = Cm[nd]
        rem = cap - fill
        mx = bins.max(0)  # [slots, buckets]
        inc = np.maximum(0, bins + v[None, None, :] - mx[None, :, :]).sum(-1)
        inc = np.where(rem > 0, inc, 1 << 40).astype(np.float64)
        ci, si = np.unravel_index(np.argmin(inc - 0.001 * rem), inc.shape)
        core_of[nd] = ci
        slot_of[nd] = si
        row_of[nd] = fill[ci, si]
        bins[ci, si] += v
        fill[ci, si] += 1

    core = core_of[t]
    j = slot_of[t]  # dest tile slot
    tin = row_of[t]
    run_len = bins.max(axis=0)  # [n_tiles, n_buckets]

    # small leading groups prime the gather/compute pipeline quickly and
    # a small trailing group shortens the tail; large steady-state groups
    # amortize the ~2.3us fixed cost per gather instruction
    sizes = [2, 2, 4]
    while sum(sizes) < n_tiles - 2:
        sizes.append(min(tiles_per_group, n_tiles - 2 - sum(sizes)))
    sizes.append(n_tiles - sum(sizes))
    groups = []
    at = 0
    for s in sizes:
        groups.append(list(range(at, at + s)))
        at += s
    n_groups = len(groups)
    grp_of_tile = np.zeros(n_tiles, dtype=np.int64)
    for g, tiles_g in enumerate(groups):
        for jj in tiles_g:
            grp_of_tile[jj] = g

    # Within a span the packing order of the group's runs is free, and
    # the number of PE matmul segments (columns touched) depends on the
    # cumulative offsets mod 128. The position after packing a subset is
    # determined by its length sum, so an exact DP over bitmasks finds
    # the order minimizing segments — at zero gather cost.
    def _best_order(lens):
        m = len(lens)
        full = 1 << m
        INF = 1 << 30
        cost = [INF] * full
        parent = [-1] * full
        cost[0] = 0
        psum = [0] * full
        for mask in range(1, full):
            lo = mask & -mask
            psum[mask] = psum[mask ^ lo] + lens[lo.bit_length() - 1]
        for mask in range(full):
            c = cost[mask]
            if c == INF:
                continue
            pos = psum[mask] % P
            for i in range(m):
                bit = 1 << i
                if mask & bit:
                    continue
                L = lens[i]
                # columns touched = floor((pos+L-1)/P) - floor(pos/P) + 1
                step = (pos + L - 1) // P - pos // P + 1 if L else 0
                nm = mask | bit
                if c + step < cost[nm]:
                    cost[nm] = c + step
                    parent[nm] = i
        order = []
        mask = full - 1
        while mask:
            i = parent[mask]
            order.append(i)
            mask ^= 1 << i
        order.reverse()
        return order

    # Stream layout: spans (g, b) hold the packed runs of group g's tiles
    # in bucket b, padded to a multiple of P at span end (pad slots
    # gather row 0; their one-hot columns are all-zero).
    spans = []  # (g, b, start, length)
    run_start = np.zeros((n_tiles, n_buckets), dtype=np.int64)
    # seg_lists[jj]: ordered (seg_id, col, b); segments are full columns
    seg_lists = [[] for _ in range(n_tiles)]
    seg_base = np.full((n_tiles, n_buckets), -1, dtype=np.int64)
    pos = 0
    for g, tiles_g in enumerate(groups):
        for bb in range(n_buckets):
            s0 = pos
            lens = [int(run_len[jj, bb]) for jj in tiles_g]
            order = _best_order(lens)
            for ti in order:
                jj = tiles_g[ti]
                run_start[jj, bb] = pos
                pos += lens[ti]
            pos = -(-pos // P) * P
            spans.append((g, bb, s0, pos - s0))
    e_pad = pos
    n_cols = e_pad // P
    n_seg_est = sum(
        (int(run_start[jj, bb]) + int(run_len[jj, bb]) - 1) // P
        - int(run_start[jj, bb]) // P + 1
        for jj in range(n_tiles) for bb in range(n_buckets)
        if run_len[jj, bb]
    )
    print(f"[kernel] e_pad={e_pad} est_segs={n_seg_est}", flush=True)

    # Assign segment ids in (group -> tile -> bucket -> col) order so each
    # group's one-hot blocks are contiguous and PSUM chains are per-tile.
    n_segs = 0
    group_seg = []  # (seg_start, seg_end) per group
    for g, tiles_g in enumerate(groups):
        g0 = n_segs
        for jj in tiles_g:
            for bb in range(n_buckets):
                rl = int(run_len[jj, bb])
                if rl == 0:
                    continue
                rs = int(run_start[jj, bb])
                c0 = rs // P
                c1 = -(-(rs + rl) // P)
                seg_base[jj, bb] = n_segs
                for c in range(c0, c1):
                    seg_lists[jj].append((n_segs, c, bb))
                    n_segs += 1
        group_seg.append((g0, n_segs))

    # Per-core data arrays
    per_core = []
    order_key = (((core * n_groups * n_buckets) + grp_of_tile[j] * n_buckets + b)
                 * n_tiles + j)
    sort_idx = np.argsort(order_key, kind="stable")
    for c in range(n_cores):
        sel = sort_idx[core[sort_idx] == c]
        jj = j[sel]
        bb2 = b[sel]
        # rank within (tile,bucket) run, order of appearance
        key = jj * n_buckets + bb2
        change = np.r_[True, key[1:] != key[:-1]]
        grp_id = np.cumsum(change) - 1
        first_pos = np.nonzero(change)[0]
        within = np.arange(len(sel)) - first_pos[grp_id]
        posn = run_start[jj, bb2] + within

        gi = np.zeros(e_pad, dtype=np.int16)
        gi[posn] = gidx_all[sel]

        # wrap gather indices: per span, index l -> [l%16, l//16], tiled x8
        # (replicated so any Q7 core pair / queue can read its copy)
        wrap = np.zeros((P, e_pad // 16), dtype=np.int16)
        for (_g, _b, s0, ln) in spans:
            w0 = s0 // 16
            seg = gi[s0:s0 + ln].reshape(ln // 16, 16).T  # [16, ln/16]
            wrap[:, w0:w0 + ln // 16] = np.tile(seg, (8, 1))

        # host-built one-hot blocks: oh[p, s*128 + f] = 1 iff this core
        # has an edge at slot (col_of_seg s, partition p) with dest row f
        s_e = seg_base[jj, bb2] + (posn // P - run_start[jj, bb2] // P)
        oh = np.zeros((P, n_segs * P), dtype=np.dtype(mybir.dt.np(FP8)))
        oh[posn % P, s_e * P + tin[sel]] = np.float32(1.0)

        # dis value per dest row (for the output scale stage)
        disF = np.ones((P, n_tiles), dtype=np.float32)
        mine = np.nonzero(core_of == c)[0]
        disF[row_of[mine], slot_of[mine]] = dis[mine]

        per_core.append({"gidx": wrap, "oh": oh, "disv": disF})

    # node -> row of the concatenated per-core y output (for unpermute)
    y_pos = core_of * npc + slot_of * P + row_of

    schedule = {
        "n": n, "d": d, "npc": npc, "n_tiles": n_tiles, "n_cols": n_cols,
        "e_pad": e_pad, "bucket": bucket, "n_buckets": n_buckets,
        "groups": groups, "spans": spans, "seg_lists": seg_lists,
        "n_segs": n_segs, "group_seg": group_seg, "y_pos": y_pos,
    }
    return schedule, per_core, {"xs": xs}


def _build_program(sched, n_cores):
    n, d, npc = sched["n"], sched["d"], sched["npc"]
    n_tiles, e_pad = sched["n_tiles"], sched["e_pad"]
    bucket, n_buckets = sched["bucket"], sched["n_buckets"]
    groups, spans, seg_lists = sched["groups"], sched["spans"], sched["seg_lists"]
    n_segs, group_seg = sched["n_segs"], sched["group_seg"]

    nc = bacc.Bacc("TRN2", target_bir_lowering=False, debug=False,
                   num_devices=n_cores, num_swdge_queues=4)
    f32 = mybir.dt.float32
    xs_d = nc.dram_tensor("xs", [n, d], BF16, kind="ExternalInput")
    gidx_d = nc.dram_tensor("gidx", [P, e_pad // 16], mybir.dt.int16,
                            kind="ExternalInput")
    oh_d = nc.dram_tensor("oh", [P, n_segs * P], FP8, kind="ExternalInput")
    disv_d = nc.dram_tensor("disv", [P, n_tiles], f32, kind="ExternalInput")
    y_d = nc.dram_tensor("y", [npc, d], f32, kind="ExternalOutput")

    nc.gpsimd.load_library(mlp_lib)

    max_span = max(ln for (_g, _b, _s, ln) in spans)
    span_by_gb = {(g, b): (s0, ln) for (g, b, s0, ln) in spans}
    group_range = []  # stream range of each group
    for g in range(len(groups)):
        gs0 = span_by_gb[(g, 0)][0]
        last_s0, last_ln = span_by_gb[(g, n_buckets - 1)]
        group_range.append((gs0, last_s0 + last_ln))
    max_gsegs = max(s1 - s0 for (s0, s1) in group_seg)

    relu = mybir.ActivationFunctionType.Relu

    with tile.TileContext(nc) as tc:
        with (
            tc.tile_pool(name="const", bufs=1) as cpool,
            tc.tile_pool(name="gidx", bufs=len(groups) + 1) as ipool,
            tc.tile_pool(name="gather", bufs=16) as gpool,
            tc.tile_pool(name="onehot", bufs=3) as opool,
            tc.tile_pool(name="psum", bufs=8, space="PSUM") as ppool,
            tc.tile_pool(name="outs", bufs=6) as ypool,
        ):
            # Q7 IRAM warmup: one tiny gather per SWDGE queue so the
            # dma_gather ucode is resident on all 4 core pairs before the
            # real gathers arrive.
            widx = cpool.tile([P, 1], mybir.dt.int16, tag="widx")
            nc.vector.memset(widx[:], 0)
            for q in range(4):
                wt = cpool.tile([P, d], BF16, tag=f"warm{q}")
                wt_3d = wt[:, :].rearrange("p (c d) -> p c d", d=d)
                nc.gpsimd.dma_gather(wt_3d, xs_d[0:bucket, :], widx[:, :],
                                     16, 16, d, single_packet=True,
                                     queue_num=q)

            disv_t = cpool.tile([P, n_tiles], f32, tag="disv")
            nc.sync.dma_start(disv_t[:], disv_d[:, :])

            # per-group gather index tiles (first group's load lands fast)
            gidx_tiles = []
            for g, (gs0, gs1) in enumerate(group_range):
                it = ipool.tile([P, (gs1 - gs0) // 16], mybir.dt.int16,
                                tag="gidx", name=f"gidx{g}")
                nc.sync.dma_start(it[:], gidx_d[:, gs0 // 16:gs1 // 16])
                gidx_tiles.append(it)

            for g, tiles_g in enumerate(groups):
                gs0, _gs1 = group_range[g]
                seg0, seg1 = group_seg[g]

                gtiles = {}
                for b in range(n_buckets):
                    s0, ln = span_by_gb[(g, b)]
                    if ln == 0:
                        continue
                    base = b * bucket
                    gt = gpool.tile([P, (max_span // P) * d], BF16, tag="gt",
                                    name=f"gt{g}_{b}")
                    gt_3d = gt[:, :(ln // P) * d].rearrange(
                        "p (c d) -> p c d", d=d
                    )
                    nc.gpsimd.dma_gather(
                        gt_3d,
                        xs_d[base:base + bucket, :],
                        gidx_tiles[g][:, (s0 - gs0) // 16:(s0 - gs0 + ln) // 16],
                        ln, ln, d,
                        single_packet=(ln <= 1024),
                        queue_num=b % 4,
                    )
                    gtiles[b] = (gt, s0)

                # host-built one-hot blocks for this group's segments
                oh = opool.tile([P, max_gsegs * P], FP8, tag="oh",
                                name=f"oh{g}")
                nc.sync.dma_start(oh[:, :(seg1 - seg0) * P],
                                  oh_d[:, seg0 * P:seg1 * P])

                for jj in tiles_g:
                    segs = seg_lists[jj]
                    rows = min(P, npc - jj * P)
                    yt = ypool.tile([P, d], f32, tag="yt", name=f"yt{jj}")
                    if segs:
                        pt = ppool.tile([P, d], f32, tag="ps",
                                        name=f"ps{jj}")
                        for si, (sid, col, b) in enumerate(segs):
                            gt, s0 = gtiles[b]
                            col_l = col - s0 // P
                            nc.tensor.matmul(
                                pt[:],
                                lhsT=oh[:, (sid - seg0) * P:(sid - seg0 + 1) * P],
                                rhs=gt[:, col_l * d:(col_l + 1) * d],
                                start=(si == 0),
                                stop=(si == len(segs) - 1),
                            )
                        nc.scalar.activation(yt[:], pt[:], relu,
                                             scale=disv_t[:, jj:jj + 1])
                    else:
                        nc.vector.memset(yt[:], 0.0)
                    nc.sync.dma_start(y_d[jj * P:jj * P + rows, :],
                                      yt[:rows, :])

    nc.compile()
    return nc


def _run(x, h, t, n_cores=8, n_buckets=4, tiles_per_group=8, trace=False):
    import time
    t0 = time.monotonic()
    sched, per_core, shared = _preprocess(x, h, t, n_cores, n_buckets,
                                          tiles_per_group)
    t1 = time.monotonic()
    print(f"[kernel] preprocess {t1 - t0:.1f}s  e_pad={sched['e_pad']} "
          f"segs={sched['n_segs']}", flush=True)
    nc = _build_program(sched, n_cores)
    t2 = time.monotonic()
    print(f"[kernel] build+tile-schedule {t2 - t1:.1f}s", flush=True)
    in_maps = [
        {"xs": shared["xs"], "gidx": pc["gidx"], "oh": pc["oh"],
         "disv": pc["disv"]}
        for pc in per_core
    ]
    res = run_bass_kernel_spmd(nc, in_maps, core_ids=list(range(n_cores)),
                               trace=trace)
    t3 = time.monotonic()
    print(f"[kernel] compile+run {t3 - t2:.1f}s", flush=True)
    y_cat = np.concatenate([res.results[c]["y"] for c in range(n_cores)],
                           axis=0)
    y = y_cat[sched["y_pos"]]
    return y, res


def kernel(x, h, t):
    y, _ = _run(np.asarray(x), np.asarray(h), np.asarray(t))
    return y
